# revision 16
# baseline (speedup 1.0000x reference)
"""Trainium2 Bass kernel for nn_ATAB_89859305767670 (dilated-conv QKV + row attention).

Sharding: data-parallel over batch B=8 -> one batch per NeuronCore, no
collectives. Each core computes its full [H,W,F] output slab.

Design (per core; W=256, C=F=64, H=128):
  - X host-prepped to [128, H+4, W+4]: partitions 0-63 = channel-major X
    shifted so padded row j holds X[j-2]; partitions 64-127 hold X[j].
    One K=128 matmul evaluates conv taps (dh=-2, dh=0) together
    (host-stacked weights); dh=+2 is a K=64 matmul on the lower half.
  - q and v convs are fused into one M=128 matmul ([Wq | Wv] stacked
    along the output dim): q lands on PSUM partitions 0-63, v on 64-127.
    k conv runs separately (M=64). 12 matmuls of N=512 per row-pair total.
  - scores are computed TRANSPOSED: S^T[kj, qi] via lhsT=kT-slice,
    rhs=qT. exp(S^T) (no max subtraction: |S|<~70 << 88, fp32-safe)
    directly yields P^T, which is exactly the moving operand the AV
    matmul needs -- no P transposes at all.
  - v^T is PE-transposed to natural [kj, F] and augmented with a ones
    column; AV = [v | 1]^T-blocks stationary, P^T moving -> out^T [F,qi]
    with the softmax denominator l[qi] appearing as row 64.
  - out^T+l are PE-transposed back to natural; DVE computes 1/l and
    scales; result DMA'd out. Output stays un-normalized until the very
    last step, so no accumulator reads and no [1,N]-broadcasts needed.
  - dtypes: conv/S inputs are fp16 (1 PE cycle/row, half-cost weight
    loads, ~tf32-grade effective precision); P^T/AV/final transposes are
    float32r because exp(S) reaches ~2e32 (far beyond fp16 range but
    fp32-safe since max S ~ 74 < 88). Measured end-to-end: max-rel err
    ~6.5e-3, resid_var ~3e-6 vs the fp32 reference.
  - v natural layout comes from fp16 DMA-transposes (XBAR) instead of PE
    transposes; the two per-row output DMAs are merged into one.
"""
import sys

sys.path.insert(0, "/opt/trn_rl_repo")

import numpy as np

B, H, W, C, F = 8, 128, 256, 64, 64
PADW = W + 4

_built = {}


def _build(nrows):
    import concourse.tile as tile
    from concourse import bacc, mybir
    from concourse.masks import make_identity

    f32, f32r = mybir.dt.float32, mybir.dt.float32r
    f16 = mybir.dt.float16
    padr = nrows + 4

    nc = bacc.Bacc("TRN2", target_bir_lowering=False, debug=False)

    xp_d = nc.dram_tensor("xp", [128, padr, PADW], f16, kind="ExternalInput").ap()
    # fused q|v pair/single weights and k pair/single weights
    wqv_p_d = nc.dram_tensor("wqv_p", [128, 3, 128], f16, kind="ExternalInput").ap()
    wqv_s_d = nc.dram_tensor("wqv_s", [C, 3, 128], f16, kind="ExternalInput").ap()
    wk_p_d = nc.dram_tensor("wk_p", [128, 3, F], f16, kind="ExternalInput").ap()
    wk_s_d = nc.dram_tensor("wk_s", [C, 3, F], f16, kind="ExternalInput").ap()
    bqv_d = nc.dram_tensor("bqv", [128, 1], f32, kind="ExternalInput").ap()
    bk_d = nc.dram_tensor("bk", [F, 1], f32, kind="ExternalInput").ap()
    ones_d = nc.dram_tensor("ones", [128, 2, 1], f32r, kind="ExternalInput").ap()
    zrows_d = nc.dram_tensor("zrows", [F, W], f32r, kind="ExternalInput").ap()
    out_d = nc.dram_tensor("out", [nrows, W, F], f32, kind="ExternalOutput").ap()

    with tile.TileContext(nc) as tc:
        with tc.tile_pool(name="const", bufs=1) as const, \
             tc.tile_pool(name="qkv", bufs=2) as sbq, \
             tc.tile_pool(name="work", bufs=6) as sbw, \
             tc.tile_pool(name="psc", bufs=2, space="PSUM") as psc, \
             tc.tile_pool(name="psk", bufs=1, space="PSUM") as psk, \
             tc.tile_pool(name="pss", bufs=2, space="PSUM") as pss, \
             tc.tile_pool(name="psa", bufs=3, space="PSUM") as psa:

            xp = const.tile([128, padr, PADW], f16, tag="xp")
            nck = 8
            step = (padr + nck - 1) // nck
            for ckk in range(nck):
                r0, r1 = ckk * step, min(padr, (ckk + 1) * step)
                if r0 < r1:
                    nc.gpsimd.dma_start(xp[:, r0:r1, :], xp_d[:, r0:r1, :])

            wqv_p = const.tile([128, 3, 128], f16, tag="wqvp")
            nc.gpsimd.dma_start(wqv_p[:], wqv_p_d[:])
            wqv_s = const.tile([C, 3, 128], f16, tag="wqvs")
            nc.gpsimd.dma_start(wqv_s[:], wqv_s_d[:])
            wk_p = const.tile([128, 3, F], f16, tag="wkp")
            nc.gpsimd.dma_start(wk_p[:], wk_p_d[:])
            wk_s = const.tile([C, 3, F], f16, tag="wks")
            nc.gpsimd.dma_start(wk_s[:], wk_s_d[:])
            bqv_t = const.tile([128, 1], f32, tag="bqv")
            nc.gpsimd.dma_start(bqv_t[:], bqv_d[:])
            bk_t = const.tile([F, 1], f32, tag="bk")
            nc.gpsimd.dma_start(bk_t[:], bk_d[:])
            ones_t = const.tile([128, 2, 1], f32r, tag="ones")
            zrows_t = const.tile([F, W], f32r, tag="zrows")
            nc.gpsimd.dma_start(zrows_t[:], zrows_d[:])
            nc.gpsimd.dma_start(ones_t[:], ones_d[:])

            ident32 = const.tile([128, 128], f32, tag="id32")
            make_identity(nc, ident32[:])
            ident = const.tile([128, 128], f32r, tag="idr")
            nc.vector.tensor_copy(ident[:], ident32[:])
            ident16 = const.tile([128, 128], f16, tag="id16")
            nc.vector.tensor_copy(ident16[:], ident32[:])

            for hp in range(nrows // 2):
                h = 2 * hp
                # ---- fused q|v conv (M=128) and k conv (M=64) ----
                cqv = psc.tile([128, 2, W], f32, tag="cqv")
                ck = psk.tile([F, 2, W], f32, tag="ck")
                for d in range(3):
                    nc.tensor.matmul(
                        cqv[:], wqv_p[:, d, :], xp[:, h:h + 2, 2 * d:2 * d + W],
                        start=(d == 0), stop=False)
                for d in range(3):
                    nc.tensor.matmul(
                        cqv[:], wqv_s[:, d, :],
                        xp[0:C, h + 4:h + 6, 2 * d:2 * d + W],
                        start=False, stop=(d == 2))
                for d in range(3):
                    nc.tensor.matmul(
                        ck[:], wk_p[:, d, :], xp[:, h:h + 2, 2 * d:2 * d + W],
                        start=(d == 0), stop=False)
                for d in range(3):
                    nc.tensor.matmul(
                        ck[:], wk_s[:, d, :],
                        xp[0:C, h + 4:h + 6, 2 * d:2 * d + W],
                        start=False, stop=(d == 2))

                qvs = sbq.tile([128, 2, W], f16, tag="qvs")
                ks_ = sbq.tile([F, 2, W], f16, tag="ks")
                for rr in range(2):
                    nc.scalar.activation(
                        qvs[:, rr, :], cqv[:, rr, :],
                        mybir.ActivationFunctionType.Identity, bias=bqv_t[:])
                    nc.scalar.activation(
                        ks_[:, rr, :], ck[:, rr, :],
                        mybir.ActivationFunctionType.Identity, bias=bk_t[:])

                for hh in range(2):
                    # ---- S^T[kj, qi] (K=F=64) ----
                    sp = pss.tile([128, 2, W], f32, tag="s")
                    for kb in range(2):
                        nc.tensor.matmul(
                            sp[:, kb, :], ks_[:, hh, 128 * kb:128 * (kb + 1)],
                            qvs[0:C, hh, :], start=True, stop=True)

                    # P^T = exp(S^T), one ACT op over [128, 512]
                    pts = sbw.tile([128, 2, W], f32r, tag="pts")
                    for kb in range(2):
                        nc.scalar.activation(
                            pts[:, kb, :], sp[:, kb, :],
                            mybir.ActivationFunctionType.Exp)

                    # ---- v natural [kj, F] via DMA transpose (fp16, XBAR) ----
                    vt16 = sbw.tile([128, 2, F], f16, tag="vt16")
                    for jb in range(2):
                        nc.sync.dma_start(
                            vt16[:, jb, :],
                            qvs[C:128, hh, 128 * jb:128 * (jb + 1)],
                            transpose=True)
                    # stationary blocks [v | 1]: col 64 = ones -> the
                    # denominator l appears as out^T row 64.
                    vts = sbw.tile([128, 2, F + 1], f32r, tag="vts")
                    nc.vector.tensor_copy(vts[:, :, 0:F], vt16[:])
                    nc.vector.tensor_copy(vts[:, :, F:F + 1], ones_t[:])

                    # ---- AV: out^T rows 0-63, denominator l at row 64 ----
                    avp = psa.tile([F + 1, W], f32, tag="misc")
                    for kb in range(2):
                        nc.tensor.matmul(
                            avp[:], vts[:, kb, :], pts[:, kb, :],
                            start=(kb == 0), stop=(kb == 1))
                    ots = sbw.tile([128, W], f32r, tag="ots")
                    if 2 * hp + hh < 3:
                        # first pass over each of the 3 pool slots: zero rows
                        # 64-127 once so the [128,128] transposes below read
                        # defined data (cols 65-127 of op are unused; row 64
                        # is overwritten by the copy that follows)
                        nc.vector.tensor_copy(ots[F:128, :], zrows_t[:])
                    for qb in range(2):
                        nc.vector.tensor_copy(ots[0:F + 1, 128 * qb:128 * (qb + 1)],
                                              avp[:, 128 * qb:128 * (qb + 1)])

                    # ---- back to natural [qi, 128] (col 64 = l), normalize ----
                    op = psa.tile([128, 2, 128], f32r, tag="misc")
                    for qb in range(2):
                        nc.tensor.transpose(
                            op[:, qb, :], ots[:, 128 * qb:128 * (qb + 1)],
                            ident[:])
                    rinv = sbw.tile([128, 2], f32, tag="rinv")
                    os_ = sbw.tile([128, 2, F], f32, tag="os")
                    for qb in range(2):
                        nc.vector.reciprocal(rinv[:, qb:qb + 1], op[:, qb, F:F + 1])
                        nc.vector.tensor_scalar_mul(
                            os_[:, qb, :], op[:, qb, 0:F], rinv[:, qb:qb + 1])
                    nc.sync.dma_start(
                        out_d[h + hh, :, :].rearrange("(b p) f -> p b f", b=2),
                        os_[:])

    nc.compile()
    return nc


def _get_nc(nrows):
    if nrows not in _built:
        _built[nrows] = _build(nrows)
    return _built[nrows]


def _host_prep(X, Wq, bq, Wk, bk, Wv, bv, nrows):
    """Build per-core input maps. X: [B, nrows, W, C] fp32, weights HWIO."""
    X = np.asarray(X, np.float32)
    Wq, Wk, Wv = (np.asarray(w, np.float32) for w in (Wq, Wk, Wv))
    bq, bk, bv = (np.asarray(b, np.float32) for b in (bq, bk, bv))
    padr = nrows + 4
    wqv_p = np.empty((128, 3, 128), np.float32)
    wqv_s = np.empty((C, 3, 128), np.float32)
    wk_p = np.empty((128, 3, F), np.float32)
    wk_s = np.empty((C, 3, F), np.float32)
    for d in range(3):
        wqv_p[0:C, d, 0:F] = Wq[0, d]
        wqv_p[0:C, d, F:128] = Wv[0, d]
        wqv_p[C:128, d, 0:F] = Wq[1, d]
        wqv_p[C:128, d, F:128] = Wv[1, d]
        wqv_s[:, d, 0:F] = Wq[2, d]
        wqv_s[:, d, F:128] = Wv[2, d]
        wk_p[0:C, d] = Wk[0, d]
        wk_p[C:128, d] = Wk[1, d]
        wk_s[:, d] = Wk[2, d]
    bqv = np.concatenate([bq, bv]).astype(np.float32).reshape(128, 1)
    bkv = np.asarray(bk, np.float32).reshape(F, 1)
    ones = np.ones((128, 2, 1), np.float32)
    zrows = np.zeros((F, W), np.float32)

    in_maps = []
    for b in range(X.shape[0]):
        xt = np.ascontiguousarray(X[b].transpose(2, 0, 1))  # [C, nrows, W]
        xpad = np.zeros((128, padr, PADW), np.float16)
        xpad[0:C, 2:2 + nrows, 2:2 + W] = xt   # lower: index j -> X[j-2]
        xpad[C:128, 0:nrows, 2:2 + W] = xt     # upper: index j -> X[j]
        in_maps.append({"xp": xpad, "wqv_p": wqv_p.astype(np.float16),
                        "wqv_s": wqv_s.astype(np.float16),
                        "wk_p": wk_p.astype(np.float16),
                        "wk_s": wk_s.astype(np.float16), "bqv": bqv, "bk": bkv,
                        "ones": ones, "zrows": zrows})
    return in_maps


def kernel(X, Wq, bq, Wk, bk, Wv, bv):
    from concourse.bass_utils import run_bass_kernel_spmd

    X = np.asarray(X, np.float32)
    nb, nrows = X.shape[0], X.shape[1]
    nc = _get_nc(nrows)
    in_maps = _host_prep(X, Wq, bq, Wk, bk, Wv, bv, nrows)
    res = run_bass_kernel_spmd(nc, in_maps, list(range(nb)))
    return np.stack([res.results[b]["out"] for b in range(nb)], axis=0)


# revision 17
# speedup vs baseline: 1.0205x; 1.0205x over previous
"""Trainium2 Bass kernel for nn_ATAB_89859305767670 (dilated-conv QKV + row attention).

Sharding: data-parallel over batch B=8 -> one batch per NeuronCore, no
collectives. Each core computes its full [H,W,F] output slab.

Design (per core; W=256, C=F=64, H=128):
  - X host-prepped to [128, H+4, W+4]: partitions 0-63 = channel-major X
    shifted so padded row j holds X[j-2]; partitions 64-127 hold X[j].
    One K=128 matmul evaluates conv taps (dh=-2, dh=0) together
    (host-stacked weights); dh=+2 is a K=64 matmul on the lower half.
  - q and v convs are fused into one M=128 matmul ([Wq | Wv] stacked
    along the output dim): q lands on PSUM partitions 0-63, v on 64-127.
    k conv runs separately (M=64). 12 matmuls of N=512 per row-pair total.
  - scores are computed TRANSPOSED: S^T[kj, qi] via lhsT=kT-slice,
    rhs=qT. exp(S^T) (no max subtraction: |S|<~70 << 88, fp32-safe)
    directly yields P^T, which is exactly the moving operand the AV
    matmul needs -- no P transposes at all.
  - v^T is PE-transposed to natural [kj, F] and augmented with a ones
    column; AV = [v | 1]^T-blocks stationary, P^T moving -> out^T [F,qi]
    with the softmax denominator l[qi] appearing as row 64.
  - out^T+l are PE-transposed back to natural; DVE computes 1/l and
    scales; result DMA'd out. Output stays un-normalized until the very
    last step, so no accumulator reads and no [1,N]-broadcasts needed.
  - dtypes: conv/S inputs are fp16 (1 PE cycle/row, half-cost weight
    loads, ~tf32-grade effective precision); P^T/AV/final transposes are
    float32r because exp(S) reaches ~2e32 (far beyond fp16 range but
    fp32-safe since max S ~ 74 < 88). Measured end-to-end: max-rel err
    ~6.5e-3, resid_var ~3e-6 vs the fp32 reference.
  - v natural layout comes from fp16 DMA-transposes (XBAR) instead of PE
    transposes; the two per-row output DMAs are merged into one.
"""
import sys

sys.path.insert(0, "/opt/trn_rl_repo")

import numpy as np

B, H, W, C, F = 8, 128, 256, 64, 64
PADW = W + 4

_built = {}


def _build(nrows):
    import concourse.tile as tile
    from concourse import bacc, mybir
    from concourse.masks import make_identity

    f32, f32r = mybir.dt.float32, mybir.dt.float32r
    f16 = mybir.dt.float16
    padr = nrows + 4

    nc = bacc.Bacc("TRN2", target_bir_lowering=False, debug=False)

    xp_d = nc.dram_tensor("xp", [128, padr, PADW], f16, kind="ExternalInput").ap()
    # fused q|v pair/single weights and k pair/single weights
    wqv_p_d = nc.dram_tensor("wqv_p", [128, 3, 128], f16, kind="ExternalInput").ap()
    wqv_s_d = nc.dram_tensor("wqv_s", [C, 3, 128], f16, kind="ExternalInput").ap()
    wk_p_d = nc.dram_tensor("wk_p", [128, 3, F], f16, kind="ExternalInput").ap()
    wk_s_d = nc.dram_tensor("wk_s", [C, 3, F], f16, kind="ExternalInput").ap()
    bqv_d = nc.dram_tensor("bqv", [128, 1], f32, kind="ExternalInput").ap()
    bk_d = nc.dram_tensor("bk", [F, 1], f32, kind="ExternalInput").ap()
    ones_d = nc.dram_tensor("ones", [128, 2, 1], f32r, kind="ExternalInput").ap()
    zrows_d = nc.dram_tensor("zrows", [F, W], f32r, kind="ExternalInput").ap()
    out_d = nc.dram_tensor("out", [nrows, W, F], f32, kind="ExternalOutput").ap()

    with tile.TileContext(nc) as tc:
        with tc.tile_pool(name="const", bufs=1) as const, \
             tc.tile_pool(name="qkv", bufs=3) as sbq, \
             tc.tile_pool(name="work", bufs=6) as sbw, \
             tc.tile_pool(name="psc", bufs=2, space="PSUM") as psc, \
             tc.tile_pool(name="psk", bufs=1, space="PSUM") as psk, \
             tc.tile_pool(name="pss", bufs=2, space="PSUM") as pss, \
             tc.tile_pool(name="psa", bufs=3, space="PSUM") as psa:

            xp = const.tile([128, padr, PADW], f16, tag="xp")
            nck = 8
            step = (padr + nck - 1) // nck
            for ckk in range(nck):
                r0, r1 = ckk * step, min(padr, (ckk + 1) * step)
                if r0 < r1:
                    nc.gpsimd.dma_start(xp[:, r0:r1, :], xp_d[:, r0:r1, :])

            wqv_p = const.tile([128, 3, 128], f16, tag="wqvp")
            nc.gpsimd.dma_start(wqv_p[:], wqv_p_d[:])
            wqv_s = const.tile([C, 3, 128], f16, tag="wqvs")
            nc.gpsimd.dma_start(wqv_s[:], wqv_s_d[:])
            wk_p = const.tile([128, 3, F], f16, tag="wkp")
            nc.gpsimd.dma_start(wk_p[:], wk_p_d[:])
            wk_s = const.tile([C, 3, F], f16, tag="wks")
            nc.gpsimd.dma_start(wk_s[:], wk_s_d[:])
            bqv_t = const.tile([128, 1], f32, tag="bqv")
            nc.gpsimd.dma_start(bqv_t[:], bqv_d[:])
            bk_t = const.tile([F, 1], f32, tag="bk")
            nc.gpsimd.dma_start(bk_t[:], bk_d[:])
            ones_t = const.tile([128, 2, 1], f32r, tag="ones")
            zrows_t = const.tile([F, W], f32r, tag="zrows")
            nc.gpsimd.dma_start(zrows_t[:], zrows_d[:])
            nc.gpsimd.dma_start(ones_t[:], ones_d[:])

            ident32 = const.tile([128, 128], f32, tag="id32")
            make_identity(nc, ident32[:])
            ident = const.tile([128, 128], f32r, tag="idr")
            nc.vector.tensor_copy(ident[:], ident32[:])
            ident16 = const.tile([128, 128], f16, tag="id16")
            nc.vector.tensor_copy(ident16[:], ident32[:])

            def emit_conv(hp):
                h = 2 * hp
                # ---- fused q|v conv (M=128) and k conv (M=64) ----
                cqv = psc.tile([128, 2, W], f32, tag="cqv")
                ck = psk.tile([F, 2, W], f32, tag="ck")
                for d in range(3):
                    nc.tensor.matmul(
                        cqv[:], wqv_p[:, d, :], xp[:, h:h + 2, 2 * d:2 * d + W],
                        start=(d == 0), stop=False)
                for d in range(3):
                    nc.tensor.matmul(
                        cqv[:], wqv_s[:, d, :],
                        xp[0:C, h + 4:h + 6, 2 * d:2 * d + W],
                        start=False, stop=(d == 2))
                for d in range(3):
                    nc.tensor.matmul(
                        ck[:], wk_p[:, d, :], xp[:, h:h + 2, 2 * d:2 * d + W],
                        start=(d == 0), stop=False)
                for d in range(3):
                    nc.tensor.matmul(
                        ck[:], wk_s[:, d, :],
                        xp[0:C, h + 4:h + 6, 2 * d:2 * d + W],
                        start=False, stop=(d == 2))

                qvs = sbq.tile([128, 2, W], f16, tag="qvs")
                ks_ = sbq.tile([F, 2, W], f16, tag="ks")
                for rr in range(2):
                    nc.scalar.activation(
                        qvs[:, rr, :], cqv[:, rr, :],
                        mybir.ActivationFunctionType.Identity, bias=bqv_t[:])
                    nc.scalar.activation(
                        ks_[:, rr, :], ck[:, rr, :],
                        mybir.ActivationFunctionType.Identity, bias=bk_t[:])
                return qvs, ks_

            def emit_attn(hp, qvs, ks_):
                h = 2 * hp
                for hh in range(2):
                    # ---- S^T[kj, qi] (K=F=64) ----
                    sp = pss.tile([128, 2, W], f32, tag="s")
                    for kb in range(2):
                        nc.tensor.matmul(
                            sp[:, kb, :], ks_[:, hh, 128 * kb:128 * (kb + 1)],
                            qvs[0:C, hh, :], start=True, stop=True)

                    # P^T = exp(S^T)
                    pts = sbw.tile([128, 2, W], f32r, tag="pts")
                    for kb in range(2):
                        nc.scalar.activation(
                            pts[:, kb, :], sp[:, kb, :],
                            mybir.ActivationFunctionType.Exp)

                    # ---- v natural [kj, F] via DMA transpose (fp16, XBAR) ----
                    vt16 = sbw.tile([128, 2, F], f16, tag="vt16")
                    for jb in range(2):
                        nc.sync.dma_start(
                            vt16[:, jb, :],
                            qvs[C:128, hh, 128 * jb:128 * (jb + 1)],
                            transpose=True)
                    # stationary blocks [v | 1]: col 64 = ones -> the
                    # denominator l appears as out^T row 64.
                    vts = sbw.tile([128, 2, F + 1], f32r, tag="vts")
                    nc.vector.tensor_copy(vts[:, :, 0:F], vt16[:])
                    nc.vector.tensor_copy(vts[:, :, F:F + 1], ones_t[:])

                    # ---- AV: out^T rows 0-63, denominator l at row 64 ----
                    avp = psa.tile([F + 1, W], f32, tag="misc")
                    for kb in range(2):
                        nc.tensor.matmul(
                            avp[:], vts[:, kb, :], pts[:, kb, :],
                            start=(kb == 0), stop=(kb == 1))
                    ots = sbw.tile([128, W], f32r, tag="ots")
                    if 2 * hp + hh < 6:
                        # first pass over each pool slot: zero rows 64-127
                        # once so the [128,128] transposes read defined data
                        nc.vector.tensor_copy(ots[F:128, :], zrows_t[:])
                    for qb in range(2):
                        nc.vector.tensor_copy(ots[0:F + 1, 128 * qb:128 * (qb + 1)],
                                              avp[:, 128 * qb:128 * (qb + 1)])

                    # ---- back to natural [qi, 128] (col 64 = l), normalize ----
                    op = psa.tile([128, 2, 128], f32r, tag="misc")
                    for qb in range(2):
                        nc.tensor.transpose(
                            op[:, qb, :], ots[:, 128 * qb:128 * (qb + 1)],
                            ident[:])
                    rinv = sbw.tile([128, 2], f32, tag="rinv")
                    os_ = sbw.tile([128, 2, F], f32, tag="os")
                    for qb in range(2):
                        nc.vector.reciprocal(rinv[:, qb:qb + 1], op[:, qb, F:F + 1])
                        nc.vector.tensor_scalar_mul(
                            os_[:, qb, :], op[:, qb, 0:F], rinv[:, qb:qb + 1])
                    nc.sync.dma_start(
                        out_d[h + hh, :, :].rearrange("(b p) f -> p b f", b=2),
                        os_[:])

            # software-pipeline with 1-pair skew: convs for pair hp are
            # emitted (and scheduled) ahead of attention for pair hp-1, so
            # every attention input was produced a full pair earlier.
            prev = None
            for hp in range(nrows // 2):
                cur = emit_conv(hp)
                if prev is not None:
                    emit_attn(hp - 1, *prev)
                prev = cur
            emit_attn(nrows // 2 - 1, *prev)

    nc.compile()
    return nc


def _get_nc(nrows):
    if nrows not in _built:
        _built[nrows] = _build(nrows)
    return _built[nrows]


def _host_prep(X, Wq, bq, Wk, bk, Wv, bv, nrows):
    """Build per-core input maps. X: [B, nrows, W, C] fp32, weights HWIO."""
    X = np.asarray(X, np.float32)
    Wq, Wk, Wv = (np.asarray(w, np.float32) for w in (Wq, Wk, Wv))
    bq, bk, bv = (np.asarray(b, np.float32) for b in (bq, bk, bv))
    padr = nrows + 4
    wqv_p = np.empty((128, 3, 128), np.float32)
    wqv_s = np.empty((C, 3, 128), np.float32)
    wk_p = np.empty((128, 3, F), np.float32)
    wk_s = np.empty((C, 3, F), np.float32)
    for d in range(3):
        wqv_p[0:C, d, 0:F] = Wq[0, d]
        wqv_p[0:C, d, F:128] = Wv[0, d]
        wqv_p[C:128, d, 0:F] = Wq[1, d]
        wqv_p[C:128, d, F:128] = Wv[1, d]
        wqv_s[:, d, 0:F] = Wq[2, d]
        wqv_s[:, d, F:128] = Wv[2, d]
        wk_p[0:C, d] = Wk[0, d]
        wk_p[C:128, d] = Wk[1, d]
        wk_s[:, d] = Wk[2, d]
    bqv = np.concatenate([bq, bv]).astype(np.float32).reshape(128, 1)
    bkv = np.asarray(bk, np.float32).reshape(F, 1)
    ones = np.ones((128, 2, 1), np.float32)
    zrows = np.zeros((F, W), np.float32)

    in_maps = []
    for b in range(X.shape[0]):
        xt = np.ascontiguousarray(X[b].transpose(2, 0, 1))  # [C, nrows, W]
        xpad = np.zeros((128, padr, PADW), np.float16)
        xpad[0:C, 2:2 + nrows, 2:2 + W] = xt   # lower: index j -> X[j-2]
        xpad[C:128, 0:nrows, 2:2 + W] = xt     # upper: index j -> X[j]
        in_maps.append({"xp": xpad, "wqv_p": wqv_p.astype(np.float16),
                        "wqv_s": wqv_s.astype(np.float16),
                        "wk_p": wk_p.astype(np.float16),
                        "wk_s": wk_s.astype(np.float16), "bqv": bqv, "bk": bkv,
                        "ones": ones, "zrows": zrows})
    return in_maps


def kernel(X, Wq, bq, Wk, bk, Wv, bv):
    from concourse.bass_utils import run_bass_kernel_spmd

    X = np.asarray(X, np.float32)
    nb, nrows = X.shape[0], X.shape[1]
    nc = _get_nc(nrows)
    in_maps = _host_prep(X, Wq, bq, Wk, bk, Wv, bv, nrows)
    res = run_bass_kernel_spmd(nc, in_maps, list(range(nb)))
    return np.stack([res.results[b]["out"] for b in range(nb)], axis=0)
